# revision 22
# baseline (speedup 1.0000x reference)
"""BiGraphConv v3: batched DMA-gather + bf16 SpMM on 8 Trainium2 cores.

Structure (per core d, owns output rows [d*12500, (d+1)*12500)):
  b is converted to bf16 and replicated to every core host-side
  (NO_COLLECTIVE; measured ~300us cheaper than the on-device AllGather,
  which remains available via the flags below).
  Edges are sorted by (block of RW=224 output rows, col group, col), padded to
  128-edge chunks; per (slab, group) a bf16 gather window [128, W_CH, 128]
  is filled by dma_gather instructions (<=8 chunks = 1024 idxs each, the
  HW cap), double buffered.
  PAD_SKIP: gather instructions are aligned to (block, group) bins so pad
  slots trail; pad idxs are -1 (HW skips trailing negatives) and the true
  per-core count is fed via a Pool register loaded from a per-core counts
  tensor. Windows are memset once so skipped slots hold finite garbage
  (killed by vv=0 in the one-hot).
  Per block: accumulate chunk matmuls into PSUM y2[f=128, r=RW]
  (lhsT = gathered chunk [e,f], rhs = one-hot s_t [e,r] built on DVE by
  iota==rr * vv; MERGE_ST>1 builds s_t for several chunks with two
  tensor_tensor ops), then y2 -> bf16 SBUF, two [128,128] matmuls against
  W give out[r, f], + bias, DMA out.

kernel(**inputs) takes FULL inputs, returns FULL [100000,128] f32.
Self-contained: shapes/sharding hardcoded.
"""

import numpy as np

import concourse.bass as bass
import concourse.bacc as bacc
import concourse.mybir as mybir
import concourse.tile as tile
from concourse.bass_utils import run_bass_kernel_spmd
from concourse.library_config import mlp as _mlp_lib

import ml_dtypes

NA = 100000
NB = 100000
NE = 1600000
F = 128
N_CORES = 8
ROWS_PER_CORE = NA // N_CORES          # 12500
RW = 224                               # output rows per block
NBLK = -(-ROWS_PER_CORE // RW)         # 56 blocks per core
OUT_ROWS = NBLK * RW                   # 12544 (padded, host slices)
G = 4                                  # col groups (int16 index range)
GROUP_ROWS = NB // G                   # 25000
SHARD_ROWS = GROUP_ROWS // N_CORES     # 3125 rows per (group, core) piece
CHUNK = 128                            # edges per matmul
W_CH = 48        # gather window capacity in chunks
GCAP = 8         # max chunks (128 idxs) per dma_gather (HW cap: 1024 idxs)

NO_COLLECTIVE = True  # replicate b on host; collective left as option
ONE_QUEUE = False
NO_LOADLIB = False
MAX_SLABS = 0
GATHER_ONLY = False   # skip compute: isolate gather cost
NO_GATHER = False     # skip gathers: isolate compute cost
SORT_BY_COL = True    # sort edges by col within each (blk,g) bin
PAD_SKIP = True       # trailing -1 idxs + per-core runtime gather counts
SPLIT_COLLECTIVE = True  # 4 sub-AllGathers instead of 1
MERGE_ST = 1          # chunks per s_t build (1 = tensor_scalar per chunk)
SCRATCH = 16384       # SWDGE descriptor ring bytes (ring = SCRATCH/16 descs)

BF16 = ml_dtypes.bfloat16

LAST_RESULTS = None
LAST_SPMD_WALL_NS = None


def set_rw(rw):
    """Set the output-block row width (for A/B testing)."""
    global RW, NBLK, OUT_ROWS
    RW = rw
    NBLK = -(-ROWS_PER_CORE // RW)
    OUT_ROWS = NBLK * RW


def _host_prep(edge_rows, edge_cols, edge_vals):
    """Sort edges by (core, block, group); build slot arrays + schedule.

    SPMD runs one module on all 8 cores, so the schedule (slab layout,
    chunk counts, instruction list) is shared: per-(block,group) chunk
    counts are maxed across cores.  Each core fills its own idx/rr/vv
    slot arrays; slack slots are idx=-1 (PAD_SKIP) or gather row 0 with
    weight 0.

    Returns (sched, per_core).
    """
    rows = np.asarray(edge_rows)
    cols = np.asarray(edge_cols)
    vals = np.asarray(edge_vals)

    order = np.argsort(rows, kind="stable")
    rows = rows[order]
    cols = cols[order]
    vals = vals[order]
    core_bounds = np.searchsorted(rows, np.arange(N_CORES + 1) * ROWS_PER_CORE)

    raw = []
    cnts = np.zeros((N_CORES, NBLK * G), dtype=np.int64)
    for d in range(N_CORES):
        a, b = core_bounds[d], core_bounds[d + 1]
        r = rows[a:b] - d * ROWS_PER_CORE
        c = cols[a:b]
        v = vals[a:b]
        key = (r // RW) * G + c // GROUP_ROWS
        if SORT_BY_COL:
            o2 = np.lexsort((c, key))
        else:
            o2 = np.argsort(key, kind="stable")
        r, c, v, key = r[o2], c[o2], v[o2], key[o2]
        cnts[d] = np.bincount(key, minlength=NBLK * G)
        raw.append((r, c, v, key))

    chunks = -(-cnts.max(axis=0) // CHUNK)         # ceil of per-key max
    chunks = np.maximum(chunks, 1)                 # every (blk,g) present

    # greedy slabs: consecutive blocks; per-group chunk sum <= W_CH.
    # The first slab is capped at 2 blocks so the first compute can start
    # after a small prefetch (shorter cold-start critical path).
    slabs = []
    cur = [0]
    gsum = chunks[0:G].astype(np.int64).copy()
    for bb in range(1, NBLK):
        nxt = gsum + chunks[bb * G:(bb + 1) * G]
        cap = 2 if not slabs else NBLK
        if (nxt <= W_CH).all() and len(cur) < cap:
            cur.append(bb)
            gsum = nxt
        else:
            slabs.append(cur)
            cur = [bb]
            gsum = chunks[bb * G:(bb + 1) * G].astype(np.int64).copy()
    slabs.append(cur)

    # slot layout: for slab: for g: for blk in slab: chunks
    # gather instructions align to (blk,g) bins (<= GCAP chunks per instr)
    key_chunk_start = np.zeros(NBLK * G, dtype=np.int64)
    instrs = []       # per slab: list of (g, win_start_chunks, pieces)
                      # piece = (chunk_start, n_chunks, instr_idx, bin_key)
    blk_sched_map = {}
    pos = 0
    n_instr = 0
    piece_meta = []   # global: (bin_key, chunk_offset_in_bin, n_chunks)
    for slab in slabs:
        sl_instrs = []
        for g in range(G):
            istart = pos
            pieces = []
            for bb in slab:
                k = bb * G + g
                key_chunk_start[k] = pos
                blk_sched_map.setdefault(bb, []).append(
                    (g, pos - istart, int(chunks[k]), istart))
                nch = int(chunks[k])
                if PAD_SKIP:
                    # bin-aligned pieces so pads trail per instruction
                    maxcnt = int(cnts[:, k].max())
                    off = 0
                    while off < nch:
                        take = min(GCAP, nch - off)
                        # static idx count: up to the max real count across
                        # cores within this piece (trailing -1s are skipped,
                        # but the cost model/descgen budget follows this)
                        nidx = int(min(max(maxcnt - off * CHUNK, 1),
                                       take * CHUNK))
                        pieces.append(
                            (pos - istart + off, take, n_instr, nidx))
                        piece_meta.append((k, off, take))
                        n_instr += 1
                        off += take
                pos += nch
            if not PAD_SKIP:
                # v2-style: pack GCAP-chunk pieces across bins
                wch = pos - istart
                for o in range(0, wch, GCAP):
                    take = min(GCAP, wch - o)
                    pieces.append((o, take, n_instr, -1))
                    n_instr += 1
            sl_instrs.append((g, istart, pieces))
        instrs.append(sl_instrs)
    tot_chunks = pos
    tot_slots = tot_chunks * CHUNK

    sched = {
        "slabs": slabs, "instrs": instrs,
        "blk_sched": blk_sched_map, "tot_chunks": tot_chunks,
        "n_instr": n_instr,
    }

    per_core = []
    for d in range(N_CORES):
        r, c, v, key = raw[d]
        cnt = cnts[d]
        gstart = np.zeros(NBLK * G + 1, dtype=np.int64)
        np.cumsum(cnt, out=gstart[1:])
        rank = np.arange(len(r)) - gstart[key]
        slot = key_chunk_start[key] * CHUNK + rank

        fill = -1 if PAD_SKIP else 0
        idx_arr = np.full(tot_slots, fill, dtype=np.int16)
        rr_arr = np.zeros(tot_slots, dtype=np.float32)
        vv_arr = np.zeros(tot_slots, dtype=np.float32)
        idx_arr[slot] = (c % GROUP_ROWS).astype(np.int16)
        rr_arr[slot] = (r % RW).astype(np.float32)
        vv_arr[slot] = v

        cnt_arr = np.zeros(max(1, sched["n_instr"]), dtype=np.int32)
        if PAD_SKIP:
            for i, (k, off, take) in enumerate(piece_meta):
                real = int(min(max(cnt[k] - off * CHUNK, 0), take * CHUNK))
                if real == 0:
                    # keep >=1 idx so the ucode never sees count 0
                    s0 = (key_chunk_start[k] + off) * CHUNK
                    if idx_arr[s0] < 0:
                        idx_arr[s0] = 0
                    real = 1
                cnt_arr[i] = real

        # dma_gather idx layout: [128, tot_slots//16] with
        # [p, w] = idx_arr[w*16 + p%16]  (16-wrapped, replicated x8)
        idx16 = idx_arr.reshape(-1, 16).T            # [16, tot_slots//16]
        idx_l = np.ascontiguousarray(np.tile(idx16, (8, 1)))

        rv_np = np.float32 if MERGE_ST == 1 else BF16
        rr_l = np.ascontiguousarray(
            rr_arr.reshape(tot_chunks, CHUNK).T).astype(rv_np)
        vv_l = np.ascontiguousarray(
            vv_arr.reshape(tot_chunks, CHUNK).T).astype(rv_np)

        per_core.append({
            "idx": idx_l, "rr": rr_l, "vv": vv_l,
            "cnt": cnt_arr.reshape(1, -1),
        })
    return sched, per_core


def _build(sched):
    """Build the (shared, SPMD) Bass module from the unified schedule."""
    f32 = mybir.dt.float32
    bf16 = mybir.dt.bfloat16
    i16 = mybir.dt.int16
    i32 = mybir.dt.int32

    tot_chunks = sched["tot_chunks"]
    tot_slots = tot_chunks * CHUNK
    instrs = sched["instrs"]
    slabs = sched["slabs"]
    blk_sched = sched["blk_sched"]
    n_instr = max(1, sched["n_instr"])

    nc = bacc.Bacc("TRN2", target_bir_lowering=False, num_swdge_queues=4,
                   dynamic_dma_scratch_size=SCRATCH)
    if NO_COLLECTIVE:
        b_full = nc.declare_dram_parameter("b_full", [NB, F], bf16, isOutput=False)
        b_groups = [b_full[g * GROUP_ROWS:(g + 1) * GROUP_ROWS, :]
                    for g in range(G)]
    elif SPLIT_COLLECTIVE:
        b_shard = nc.declare_dram_parameter(
            "b_shard", [NB // N_CORES, F], bf16, isOutput=False)
        b_shard_ints = [
            nc.dram_tensor(f"b_shard_int{g}", [SHARD_ROWS, F], bf16)
            for g in range(G)]
        b_fulls = [
            nc.dram_tensor(f"b_full{g}", [GROUP_ROWS, F], bf16,
                           addr_space="Shared")
            for g in range(G)]
        b_groups = [t[:, :] for t in b_fulls]
    else:
        b_shard = nc.declare_dram_parameter(
            "b_shard", [NB // N_CORES, F], bf16, isOutput=False)
        b_shard_int = nc.dram_tensor("b_shard_int", [NB // N_CORES, F], bf16)
        b_full = nc.dram_tensor("b_full", [NB, F], bf16, addr_space="Shared")
        b_groups = [b_full[g * GROUP_ROWS:(g + 1) * GROUP_ROWS, :]
                    for g in range(G)]
    w_d = nc.declare_dram_parameter("w", [F, F], bf16, isOutput=False)
    bias_d = nc.declare_dram_parameter("bias_bcast", [128, F], f32, isOutput=False)
    rv_dt = f32 if MERGE_ST == 1 else bf16
    iota_d = nc.declare_dram_parameter("iota", [128, MERGE_ST * RW], bf16,
                                       isOutput=False)
    idx_d = nc.declare_dram_parameter(
        "idx", [128, tot_slots // 16], i16, isOutput=False)
    rr_d = nc.declare_dram_parameter("rr", [128, tot_chunks], rv_dt, isOutput=False)
    vv_d = nc.declare_dram_parameter("vv", [128, tot_chunks], rv_dt, isOutput=False)
    cnt_d = nc.declare_dram_parameter("cnt", [1, n_instr], i32, isOutput=False)
    out_d = nc.declare_dram_parameter("out", [OUT_ROWS, F], f32, isOutput=True)

    with tile.TileContext(nc) as tc:
        with (
            tc.tile_pool(name="const", bufs=1) as const_pool,
            tc.tile_pool(name="meta", bufs=1) as meta_pool,
            tc.tile_pool(name="gather", bufs=2) as gather_pool,
            tc.tile_pool(name="st", bufs=8) as st_pool,
            tc.tile_pool(name="y2sb", bufs=3) as y2sb_pool,
            tc.tile_pool(name="outsb", bufs=4) as outsb_pool,
            tc.tile_pool(name="y2ps", bufs=2, space="PSUM") as y2ps_pool,
            tc.tile_pool(name="outps", bufs=2, space="PSUM") as outps_pool,
        ):
            w_sb = const_pool.tile([F, F], bf16)
            bias_sb = const_pool.tile([128, F], f32)
            iota_sb = const_pool.tile([128, MERGE_ST * RW], bf16)
            nc.sync.dma_start(out=w_sb[:], in_=w_d[:])
            nc.sync.dma_start(out=bias_sb[:], in_=bias_d[:])
            nc.sync.dma_start(out=iota_sb[:], in_=iota_d[:])

            idx_sb = meta_pool.tile([128, tot_slots // 16], i16)
            rr_sb = meta_pool.tile([128, tot_chunks], rv_dt)
            vv_sb = meta_pool.tile([128, tot_chunks], rv_dt)
            cnt_sb = meta_pool.tile([1, n_instr], i32)
            nc.sync.dma_start(out=idx_sb[:], in_=idx_d[:])
            nc.sync.dma_start(out=rr_sb[:], in_=rr_d[:])
            nc.sync.dma_start(out=vv_sb[:], in_=vv_d[:])
            nc.sync.dma_start(out=cnt_sb[:], in_=cnt_d[:])

            if not NO_LOADLIB:
                nc.gpsimd.load_library(_mlp_lib)
            if not NO_COLLECTIVE:
                if SPLIT_COLLECTIVE:
                    for g in range(G):
                        nc.sync.dma_start(
                            out=b_shard_ints[g][:],
                            in_=b_shard[g * SHARD_ROWS:(g + 1) * SHARD_ROWS, :])
                        nc.gpsimd.collective_compute(
                            "AllGather",
                            mybir.AluOpType.bypass,
                            replica_groups=[list(range(N_CORES))],
                            ins=[b_shard_ints[g][:]],
                            outs=[b_fulls[g][:]],
                        )
                else:
                    nc.sync.dma_start(out=b_shard_int[:], in_=b_shard[:])
                    nc.gpsimd.collective_compute(
                        "AllGather",
                        mybir.AluOpType.bypass,
                        replica_groups=[list(range(N_CORES))],
                        ins=[b_shard_int[:]],
                        outs=[b_full[:]],
                    )

            cnt_regs = None
            if PAD_SKIP:
                max_pieces = max(
                    (len(p) for sl in instrs for (_, _, p) in sl), default=1)
                cnt_regs = [
                    nc.alloc_register(mybir.EngineType.Pool, f"gcnt{j}")
                    for j in range(max_pieces)]

            for s, slab in enumerate(slabs):
                if MAX_SLABS and s >= MAX_SLABS:
                    break
                gt = {}
                for (g, istart, pieces) in instrs[s]:
                    t = gather_pool.tile([128, W_CH, F], bf16, tag=f"g{g}")
                    gt[g] = (t, istart)
                    if NO_GATHER:
                        nc.gpsimd.memset(t[:], 0.25)
                        continue
                    if PAD_SKIP and s < 2:
                        nc.vector.memset(t[:], 0.0)
                    if PAD_SKIP and pieces:
                        # one Pool instruction loads all this window's counts
                        i0 = pieces[0][2]
                        npc = len(pieces)
                        nc.gpsimd.reg_load(
                            cnt_regs[:npc], cnt_sb[0:1, i0:i0 + npc])
                    for pj, (o, cnt, iidx, nidx) in enumerate(pieces):
                        n_idxs = nidx if PAD_SKIP else cnt * CHUNK
                        if PAD_SKIP:
                            reg = cnt_regs[pj]
                        else:
                            reg = n_idxs
                        nc.gpsimd.dma_gather(
                            out_ap=t[:, o:o + cnt, :],
                            in_ap=b_groups[g],
                            idxs_ap=idx_sb[:, (istart + o) * 8:(istart + o + cnt) * 8],
                            num_idxs=n_idxs,
                            num_idxs_reg=reg,
                            elem_size=F,
                            queue_num=0 if ONE_QUEUE else g % 4,
                        )
                if GATHER_ONLY:
                    continue
                for bb in slab:
                    sched_bb = blk_sched[bb]
                    tot_mm = sum(nch for (_, _, nch, _) in sched_bb)
                    y2 = y2ps_pool.tile([F, RW], f32, tag="y2")
                    mm = 0
                    for (g, wstart, nch, istart) in sched_bb:
                        t, _ = gt[g]
                        for k0 in range(0, nch, MERGE_ST):
                            mcnt = min(MERGE_ST, nch - k0)
                            gchunk = istart + wstart + k0
                            if MERGE_ST == 1:
                                s_t = st_pool.tile([128, RW], bf16, tag="s_t")
                                nc.vector.tensor_scalar(
                                    out=s_t[:],
                                    in0=iota_sb[:, 0:RW],
                                    scalar1=rr_sb[:, gchunk:gchunk + 1],
                                    scalar2=vv_sb[:, gchunk:gchunk + 1],
                                    op0=mybir.AluOpType.is_equal,
                                    op1=mybir.AluOpType.mult,
                                )
                                s_ts = [(0, s_t)]
                            else:
                                s_t = st_pool.tile(
                                    [128, mcnt, RW], bf16, tag="s_t")
                                rr_b = (rr_sb[:, gchunk:gchunk + mcnt]
                                        .unsqueeze(2).broadcast_to(
                                            [128, mcnt, RW]))
                                vv_b = (vv_sb[:, gchunk:gchunk + mcnt]
                                        .unsqueeze(2).broadcast_to(
                                            [128, mcnt, RW]))
                                iota_v = iota_sb[:, 0:mcnt * RW].rearrange(
                                    "p (m r) -> p m r", m=mcnt)
                                nc.vector.tensor_tensor(
                                    out=s_t[:], in0=iota_v, in1=rr_b,
                                    op=mybir.AluOpType.is_equal,
                                )
                                nc.vector.tensor_tensor(
                                    out=s_t[:], in0=s_t[:], in1=vv_b,
                                    op=mybir.AluOpType.mult,
                                )
                                s_ts = [(i, s_t[:, i, :]) for i in range(mcnt)]
                            for (i, st_ap) in s_ts:
                                k = k0 + i
                                nc.tensor.matmul(
                                    out=y2[:],
                                    lhsT=t[:, wstart + k, :],
                                    rhs=st_ap,
                                    start=(mm == 0),
                                    stop=(mm == tot_mm - 1),
                                )
                                mm += 1
                    y2_sb = y2sb_pool.tile([F, RW], bf16, tag="y2sb")
                    nc.scalar.activation(
                        out=y2_sb[:], in_=y2[:],
                        func=mybir.ActivationFunctionType.Copy,
                    )
                    for h0 in range(0, RW, 128):
                        hw = min(128, RW - h0)
                        o_ps = outps_pool.tile([128, F], f32, tag="ops")
                        nc.tensor.matmul(
                            out=o_ps[0:hw, :],
                            lhsT=y2_sb[:, h0:h0 + hw],
                            rhs=w_sb[:],
                            start=True, stop=True,
                        )
                        o_sb = outsb_pool.tile([128, F], f32, tag="osb")
                        nc.vector.tensor_tensor(
                            out=o_sb[0:hw, :], in0=o_ps[0:hw, :],
                            in1=bias_sb[0:hw, :],
                            op=mybir.AluOpType.add,
                        )
                        nc.sync.dma_start(
                            out=out_d[bb * RW + h0: bb * RW + h0 + hw, :],
                            in_=o_sb[0:hw, :],
                        )
    nc.finalize()
    return nc


def prepare(b_input, edge_rows, edge_cols, edge_vals, a_weight, a_bias):
    b_input = np.ascontiguousarray(np.asarray(b_input, dtype=np.float32))
    a_weight = np.ascontiguousarray(np.asarray(a_weight, dtype=np.float32))
    a_bias = np.asarray(a_bias, dtype=np.float32)

    sched, per_core = _host_prep(edge_rows, edge_cols, edge_vals)
    nc = _build(sched)

    bias_bcast = np.tile(a_bias[None, :], (128, 1)).astype(np.float32)
    iota1 = np.arange(RW, dtype=np.float32)
    iota = np.tile(np.tile(iota1, MERGE_ST)[None, :], (128, 1)).astype(BF16)
    w_bf = a_weight.astype(BF16)
    b_bf = b_input.astype(BF16)

    in_maps = []
    for d in range(N_CORES):
        m = {
            "w": w_bf,
            "bias_bcast": bias_bcast,
            "iota": iota,
            "idx": per_core[d]["idx"],
            "rr": per_core[d]["rr"],
            "vv": per_core[d]["vv"],
            "cnt": per_core[d]["cnt"],
        }
        if NO_COLLECTIVE:
            m["b_full"] = b_bf
        elif SPLIT_COLLECTIVE:
            m["b_shard"] = np.ascontiguousarray(np.concatenate(
                [b_bf[g * GROUP_ROWS + d * SHARD_ROWS:
                      g * GROUP_ROWS + (d + 1) * SHARD_ROWS]
                 for g in range(G)], axis=0))
        else:
            m["b_shard"] = b_bf[d * (NB // N_CORES):(d + 1) * (NB // N_CORES)]
        in_maps.append(m)

    def post(results):
        out = np.empty((NA, F), dtype=np.float32)
        for d in range(N_CORES):
            out[d * ROWS_PER_CORE:(d + 1) * ROWS_PER_CORE] = (
                results[d]["out"][:ROWS_PER_CORE]
            )
        return out

    return nc, in_maps, post


def kernel(b_input, edge_rows, edge_cols, edge_vals, a_weight, a_bias):
    global LAST_RESULTS, LAST_SPMD_WALL_NS
    nc, in_maps, post = prepare(
        b_input, edge_rows, edge_cols, edge_vals, a_weight, a_bias)

    import time as _time
    _t0 = _time.time()
    res = run_bass_kernel_spmd(nc, in_maps, core_ids=list(range(N_CORES)))
    LAST_SPMD_WALL_NS = int((_time.time() - _t0) * 1e9)
    LAST_RESULTS = res
    return post(res.results)
